# revision 44
# baseline (speedup 1.0000x reference)
"""Dilated attention (banded local-window attention) for Trainium2.

Problem: q,k,v [1, 16, 4096, 64] fp32; dilation r=2, window 128 (band |i-j|<=64
within each of the 2 strided subsequences of length 2048 per head).

Sharding: 16 heads -> 8 cores x 2 heads. Host work is a single fp32->bf16 cast
per tensor (ml_dtypes, ~10ms each); ALL relayout happens on device:

- Q^T/K^T: xbar DMA-transposes turn DRAM-natural [2048, (r d)=128] bf16 into
  SBUF [(r d), 2048] d-major layout; offset r=0 lands on partitions 0:64 and
  r=1 on 64:128 (the dilation de-interleave falls out of the transpose since
  (r d) becomes the partition dim). K^T lands at column offset 64 inside a
  [128, 2176] tile whose edge columns are zeroed. The transposes are SP-issued
  (ACT-issued DmaTranspose corrupts on this runtime) and serialize against
  all other DMA traffic, so they run as one tight chain before the V loads.
- V: the shifted window layout vsh[p, t, r, :64] = v[2*(128t - 64 + p) + r]
  (+ ones column for the row-sum trick), built with strided DMAs split across
  both HWDGE queues (explicitly pinned after the xbar chain) + edge memsets.

Per block, queries are tiled in 16 tiles of 128; each tile attends to a
256-key window. Scores are computed transposed (S^T[jj, i]) so probabilities
come out pre-transposed for the PV matmul. Softmax skips the max-subtraction
(scores ~ N(0,1) after the 1/8 scale) and folds the scale into the ScalarE
exp. The band mask is a 0/1 bf16 multiply after exp. Row sums come from the
ones-column: out = (P@[V|1])[:,:64] / (P@[V|1])[:,64].

The ScalarE exp chain is the serial bottleneck; emission is interleaved so
nothing ever stalls it: both heads' QK+exp run back-to-back while V loads,
head 0's PV/normalize work is woven into head 1's QK loop (2-group lag), and
the band masks are split between DVE and the otherwise-idle Pool engine so
DVE can retire normalizes as soon as each PV lands.
"""

import sys

for _p in ("/opt/trn_rl_repo", "/opt/trn_rl_repo/concourse"):
    if _p not in sys.path:
        sys.path.insert(0, _p)

import numpy as np
import ml_dtypes

import concourse.bass as bass
import concourse.mybir as mybir
import concourse.tile as tile
from concourse import bacc
from concourse.bass_utils import run_bass_kernel_spmd
from concourse.tile_rust import add_dep_helper

N_CORES = 8
B, H, S, D = 1, 16, 4096, 64
R = 2                      # dilation rate
NSEQ = S // R              # 2048 per-offset sequence length
HALF = 64                  # window//2
NT = NSEQ // 128           # 16 query tiles per block
HPC = H // N_CORES         # heads per core = 2

F32 = mybir.dt.float32
BF16 = mybir.dt.bfloat16

# xbar piece boundaries per head. QK group g reads kTp columns up to
# 256g+384, i.e. keys up to 256g+320; a piece [0:sp] covers groups with
# 256g+320 <= sp. h0 starts with a small piece so its first groups begin
# as early as possible; h1's pieces land well before its compute anyway.
_QK_PIECES = {0: (512, 1408, NSEQ), 1: (1088, NSEQ)}
# group gated on piece i (piece 0's gate is always at g=0)
_QK_GATES = {0: {1: 1, 2: 5}, 1: {1: 3}}


class _HeadState:
    __slots__ = ("qTs", "kTp", "vsh", "pms", "pos", "out_sb", "dm_gate")

    def __init__(self):
        self.pms = [None] * (NT // 2)
        self.pos = [None] * (NT // 2)
        self.dm_gate = None


def _dummy(nc, dmy, col, a, b, after=None):
    """One PE-proc absorber: a tiny dummy matmul reading cells a/b so the PE
    sequencer observes their DMA-queue semaphores here (<=2 waits). The
    scheduler floats unpinned dummies past the real matmuls (leaving the DMA
    waits guarding nothing), so callers must pin consumers after these via
    add_dep_helper, and `after` keeps the dummy from stalling the PE early."""
    dm = nc.tensor.matmul(dmy[0:1, col : col + 1], lhsT=a, rhs=b,
                          start=True, stop=True)
    if after is not None:
        add_dep_helper(dm.ins, after.ins, reason="absorb order")
    return dm


def _alloc_head(tc, pools):
    nc = tc.nc
    (trans, vpool, *_rest) = pools
    st = _HeadState()
    st.qTs = trans.tile([128, NSEQ], BF16, tag="qTs")
    st.kTp = trans.tile([128, NSEQ + 128], BF16, tag="kTp")
    st.vsh = vpool.tile([128, NT + 1, R, D + 1], BF16, tag="vsh")

    # kTp edge zeros + vsh edges/ones on Pool (keeps both DVE and the DMA
    # paths clear at startup)
    nc.gpsimd.memset(st.kTp[:, 0:HALF], 0.0)
    nc.gpsimd.memset(st.kTp[:, NSEQ + HALF :], 0.0)
    nc.gpsimd.memset(st.vsh[0:64, 0, :, 0:D], 0.0)
    nc.gpsimd.memset(st.vsh[64:128, NT, :, 0:D], 0.0)
    nc.gpsimd.memset(st.vsh[:, :, :, D], 1.0)
    return st


def _load_head_qk(tc, st, qd, kd, h):
    """Xbar transposes, first-needed first. Returns the last transpose."""
    nc = tc.nc
    lo = 0
    last = None
    for sp in _QK_PIECES[h]:
        nc.sync.dma_start_transpose(st.qTs[:, lo:sp], qd[h, lo:sp, :])
        last = nc.sync.dma_start_transpose(
            st.kTp[:, HALF + lo : HALF + sp], kd[h, lo:sp, :]
        )
        lo = sp
    return last


def _load_head_v(tc, st, vd, h, after_xbar):
    """vsh: partitions 64:128 <- tile-aligned rows, 0:64 <- next tile's rows
    (split by dilation offset r to keep each AP 3-dim for the balancer).
    Two loads ride the SP queue (FIFO behind the xbar chain), two go on the
    ACT queue pinned explicitly after the last transpose — unpinned they get
    scheduled before it and every queue switch inside the transpose
    serialization order costs a multi-microsecond hop."""
    nc = tc.nc
    for r in range(R):
        nc.sync.dma_start(
            st.vsh[64:128, 0:NT, r, 0:D],
            vd[h, :, 0:64, r].rearrange("t pp d -> pp t d"),
        )
        dm = nc.scalar.dma_start(
            st.vsh[0:64, 1 : NT + 1, r, 0:D],
            vd[h, :, 64:128, r].rearrange("t pp d -> pp t d"),
        )
        add_dep_helper(dm.ins, after_xbar.ins, reason="after xbar chain")


def _emit_qk_group(tc, pools, masks, st, dmy, h, g, ctx):
    """QK scores + exp + band mask for one group of 2 query tiles."""
    nc = tc.nc
    (trans, vpool, ppool, opool, rpool, ps_pool, po_pool, dpool) = pools
    m_first, m_mid, m_last = masks
    qTs, kTp = st.qTs, st.kTp
    pieces = _QK_PIECES[h]
    gates = _QK_GATES[h]
    dc = 8 * (h % 2)

    if g == 0:
        st.dm_gate = _dummy(
            nc, dmy, dc + 0, qTs[0:1, 0:1], qTs[0:1, HALF : HALF + 1],
            after=ctx["last_pe"],
        )
        st.dm_gate = _dummy(
            nc, dmy, dc + 1, kTp[0:1, HALF : HALF + 1],
            kTp[0:1, HALF + 1 : HALF + 2], after=st.dm_gate,
        )
    else:
        for pi, gg in gates.items():
            if g == gg:
                sp = pieces[pi - 1]
                st.dm_gate = _dummy(
                    nc, dmy, dc + 1 + pi, qTs[0:1, sp : sp + 1],
                    kTp[0:1, HALF + sp : HALF + sp + 1], after=ctx["last_pe"],
                )

    ps = ps_pool.tile([128, 2, 4, 128], F32, tag="ps")
    q0 = 256 * g
    for blk, (p0, p1) in enumerate(((0, 64), (64, 128))):
        mm = nc.tensor.matmul(
            ps[:, blk, 0, :],
            lhsT=kTp[p0:p1, q0 : q0 + 128],
            rhs=qTs[p0:p1, q0 : q0 + 128],
            start=True,
            stop=True,
        )
        if st.dm_gate is not None:
            add_dep_helper(mm.ins, st.dm_gate.ins, reason="absorb order")
            st.dm_gate = None
        nc.tensor.matmul(
            ps[:, blk, 1:3, :],
            lhsT=kTp[p0:p1, q0 + 128 : q0 + 256],
            rhs=qTs[p0:p1, q0 : q0 + 256],
            start=True,
            stop=True,
        )
        ctx["last_pe"] = nc.tensor.matmul(
            ps[:, blk, 3, :],
            lhsT=kTp[p0:p1, q0 + 256 : q0 + 384],
            rhs=qTs[p0:p1, q0 + 128 : q0 + 256],
            start=True,
            stop=True,
        )

    # exp((q.k)/8) for both tiles in one ScalarE pass; bf16 out.
    pt = ppool.tile([128, 2, 4, 128], BF16, tag="pt")
    nc.scalar.activation(
        pt[:], ps[:], mybir.ActivationFunctionType.Exp, scale=1.0 / float(D) ** 0.5
    )

    # band mask (0/1 multiply): lo segs keep i<=jj, hi segs keep i>=jj;
    # first/last tiles additionally kill out-of-range keys. Head 1's early
    # groups run on Pool (slack until their PV) so DVE can retire head 0's
    # normalizes during the exp chain instead of after it.
    eng = nc.gpsimd if (h == 1 and g <= 4) else nc.vector
    pm = ppool.tile([128, 2, 4, 128], BF16, tag="pm")
    if 0 < g < NT // 2 - 1:
        eng.tensor_tensor(
            pm[:].rearrange("p b (j c) i -> p (b j) c i", c=2),
            pt[:].rearrange("p b (j c) i -> p (b j) c i", c=2),
            m_mid[:, None, :, :].to_broadcast((128, 4, 2, 128)),
            mybir.AluOpType.mult,
        )
    else:
        for j, t in enumerate((2 * g, 2 * g + 1)):
            m = m_first if t == 0 else (m_last if t == NT - 1 else m_mid)
            eng.tensor_tensor(
                pm[:, :, 2 * j : 2 * j + 2, :],
                pt[:, :, 2 * j : 2 * j + 2, :],
                m[:, None, :, :].to_broadcast((128, 2, 2, 128)),
                mybir.AluOpType.mult,
            )
    st.pms[g] = pm


def _emit_pv_gates(tc, st, dmy, h, ctx):
    nc = tc.nc
    dc = 8 * (h % 2)
    st.dm_gate = _dummy(
        nc, dmy, dc + 5, st.vsh[64:65, 0, 0, 0:1], st.vsh[64:65, 0, 1, 0:1],
        after=ctx["last_pe"],
    )
    st.dm_gate = _dummy(
        nc, dmy, dc + 6, st.vsh[0:1, 1, 0, 0:1], st.vsh[0:1, 1, 1, 0:1],
        after=st.dm_gate,
    )


def _emit_pv_group(tc, pools, st, out, h, g, ctx):
    """PV + row-sum + normalize for one group."""
    nc = tc.nc
    (trans, vpool, ppool, opool, rpool, ps_pool, po_pool, dpool) = pools
    vsh, pm = st.vsh, st.pms[g]

    po = po_pool.tile([128, 2, 2, D + 1], F32, tag="po")
    for j, t in enumerate((2 * g, 2 * g + 1)):
        for blk in range(R):
            mm = nc.tensor.matmul(
                po[:, j, blk, :],
                lhsT=pm[:, blk, 2 * j + 0, :],
                rhs=vsh[:, t, blk, :],
                start=True,
                stop=False,
            )
            if st.dm_gate is not None:
                add_dep_helper(mm.ins, st.dm_gate.ins, reason="absorb order")
                st.dm_gate = None
            ctx["last_pe"] = nc.tensor.matmul(
                po[:, j, blk, :],
                lhsT=pm[:, blk, 2 * j + 1, :],
                rhs=vsh[:, t + 1, blk, :],
                start=False,
                stop=True,
            )
    # normalize both tiles at once: out = po[..., 0:64] / po[..., 64]
    rc = rpool.tile([128, 2, 2], F32, tag="rc")
    nc.vector.reciprocal(rc[:], po[:, :, :, D])
    nc.vector.tensor_tensor(
        st.out_sb[:, 2 * g : 2 * g + 2, :].rearrange("p t (r d) -> p t r d", r=R),
        po[:, :, :, 0:D],
        rc[:, :, :, None].to_broadcast((128, 2, R, D)),
        mybir.AluOpType.mult,
    )

    if g == NT // 2 - 1:
        # single store per head (a store interleaved with the transposes
        # would serialize against them; by now the xbar chain has drained)
        nc.sync.dma_start(
            out[h].rearrange("(t p r) d -> p t (r d)", p=128, r=R),
            st.out_sb[:],
        )


def _build_masks(tc, mpool):
    """Band masks [128, 2(lo|hi), 128].

    Element [jj, c, i]: lo (c=0) keeps i <= jj, hi (c=1) keeps i >= jj.
    t=0 variant also kills jj < 64 in lo (keys < 0); t=NT-1 variant kills
    jj >= 64 in hi (keys >= NSEQ).
    """
    nc = tc.nc
    ge = mybir.AluOpType.is_ge
    tiles = []
    for name in ("m_first", "m_mid", "m_last"):
        m = mpool.tile([128, 2, 128], BF16, tag=name)
        nc.gpsimd.memset(m[:], 1.0)
        # lo: keep jj - i >= 0
        nc.gpsimd.affine_select(
            m[:, 0, :], m[:, 0, :], [[-1, 128]], ge, 0.0,
            base=0, channel_multiplier=1,
        )
        # hi: keep i - jj >= 0
        nc.gpsimd.affine_select(
            m[:, 1, :], m[:, 1, :], [[1, 128]], ge, 0.0,
            base=0, channel_multiplier=-1,
        )
        tiles.append(m)
    m_first, m_mid, m_last = tiles
    # first tile: lo also needs jj >= 64
    nc.gpsimd.affine_select(
        m_first[:, 0, :], m_first[:, 0, :], [[0, 128]], ge, 0.0,
        base=-HALF, channel_multiplier=1,
    )
    # last tile: hi also needs jj <= 63
    nc.gpsimd.affine_select(
        m_last[:, 1, :], m_last[:, 1, :], [[0, 128]], ge, 0.0,
        base=HALF - 1, channel_multiplier=-1,
    )

    # DVE-proc absorber: make the DVE clock observe the final Pool init op
    # here so the first real DVE mask multiply carries only its exp wait
    # (the TensorTensor ISA struct has a single sync-wait slot).
    mdmy = mpool.tile([1, 2], BF16, tag="mdmy")
    nc.vector.tensor_tensor(
        mdmy[0:1, 0:1], m_last[0:1, 0, 0:1], m_last[0:1, 1, 0:1],
        mybir.AluOpType.mult,
    )
    return m_first, m_mid, m_last


def build_bass():
    nc = bacc.Bacc("TRN2", target_bir_lowering=False, debug=False)
    qd = nc.dram_tensor("q", [HPC, NSEQ, R * D], BF16, kind="ExternalInput")
    kd = nc.dram_tensor("k", [HPC, NSEQ, R * D], BF16, kind="ExternalInput")
    vd = nc.dram_tensor("v", [HPC, NT, 128, R, D], BF16, kind="ExternalInput")
    out = nc.dram_tensor("out", [HPC, S, D], F32, kind="ExternalOutput")

    NG = NT // 2
    with tile.TileContext(nc) as tc:
        with (
            tc.tile_pool(name="mpool", bufs=1) as mpool,
            tc.tile_pool(name="trans", bufs=2) as trans,
            tc.tile_pool(name="vpool", bufs=2) as vpool,
            tc.tile_pool(name="ppool", bufs=4 * NG) as ppool,
            tc.tile_pool(name="opool", bufs=2) as opool,
            tc.tile_pool(name="rpool", bufs=8) as rpool,
            tc.tile_pool(name="ps_pool", bufs=2, space="PSUM") as ps_pool,
            tc.tile_pool(name="po_pool", bufs=3, space="PSUM") as po_pool,
            tc.tile_pool(name="dmy_pool", bufs=1, space="PSUM") as dpool,
        ):
            masks = _build_masks(tc, mpool)
            pools = (trans, vpool, ppool, opool, rpool, ps_pool, po_pool, dpool)
            heads = [_alloc_head(tc, pools) for _ in range(HPC)]
            for h in range(HPC):
                out_sb = opool.tile([128, NT, 128], F32, tag="out_sb")
                heads[h].out_sb = out_sb
            # xbar chain first (it serializes against all other DMA), then
            # the V loads ride behind it
            last_xbar = None
            for h in range(HPC):
                last_xbar = _load_head_qk(tc, heads[h], qd, kd, h)
            for h in range(HPC):
                _load_head_v(tc, heads[h], vd, h, last_xbar)

            dmy = dpool.tile([1, 16], F32, tag="dmy")
            ctx = {"last_pe": None}
            # head 0: QK+exp+mask for all groups
            for g in range(NG):
                _emit_qk_group(tc, pools, masks, heads[0], dmy, 0, g, ctx)
            # head 1 QK with head-0 PV woven in at a 2-group lag (v-loads
            # have landed by then; PE never stalls the exp chain)
            for g in range(NG):
                _emit_qk_group(tc, pools, masks, heads[1], dmy, 1, g, ctx)
                if g == 2:
                    _emit_pv_gates(tc, heads[0], dmy, 0, ctx)
                if g >= 2:
                    _emit_pv_group(tc, pools, heads[0], out[:], 0, g - 2, ctx)
            for g in range(NG - 2, NG):
                _emit_pv_group(tc, pools, heads[0], out[:], 0, g, ctx)
            _emit_pv_gates(tc, heads[1], dmy, 1, ctx)
            for g in range(NG):
                _emit_pv_group(tc, pools, heads[1], out[:], 1, g, ctx)
    nc.compile()
    return nc


_NC_CACHE = None


def kernel(q: np.ndarray, k: np.ndarray, v: np.ndarray) -> np.ndarray:
    global _NC_CACHE
    if _NC_CACHE is None:
        _NC_CACHE = build_bass()
    nc = _NC_CACHE

    # host side: one bf16 cast per tensor; all relayout is on-device
    qb = np.asarray(q, dtype=np.float32).reshape(H, NSEQ, R * D).astype(
        ml_dtypes.bfloat16
    )
    kb = np.asarray(k, dtype=np.float32).reshape(H, NSEQ, R * D).astype(
        ml_dtypes.bfloat16
    )
    vb = np.asarray(v, dtype=np.float32).reshape(H, NT, 128, R, D).astype(
        ml_dtypes.bfloat16
    )

    in_maps = []
    for c in range(N_CORES):
        hs = slice(c * HPC, (c + 1) * HPC)
        in_maps.append({"q": qb[hs], "k": kb[hs], "v": vb[hs]})

    res = run_bass_kernel_spmd(nc, in_maps, core_ids=list(range(N_CORES)))
    out = np.empty((B, H, S, D), dtype=np.float32)
    for c in range(N_CORES):
        out[0, c * HPC : (c + 1) * HPC] = res.results[c]["out"]
    return out


# revision 45
# speedup vs baseline: 1.0745x; 1.0745x over previous
"""Dilated attention (banded local-window attention) for Trainium2.

Problem: q,k,v [1, 16, 4096, 64] fp32; dilation r=2, window 128 (band |i-j|<=64
within each of the 2 strided subsequences of length 2048 per head).

Sharding: 16 heads -> 8 cores x 2 heads. Host work is a single fp32->bf16 cast
per tensor (ml_dtypes, ~10ms each); ALL relayout happens on device:

- Q^T/K^T: xbar DMA-transposes turn DRAM-natural [2048, (r d)=128] bf16 into
  SBUF [(r d), 2048] d-major layout; offset r=0 lands on partitions 0:64 and
  r=1 on 64:128 (the dilation de-interleave falls out of the transpose since
  (r d) becomes the partition dim). K^T lands at column offset 64 inside a
  [128, 2176] tile whose edge columns are zeroed. The transposes are SP-issued
  (ACT-issued DmaTranspose corrupts on this runtime) and serialize against
  all other DMA traffic, so they run as one tight chain before the V loads.
- V: the shifted window layout vsh[p, t, r, :64] = v[2*(128t - 64 + p) + r]
  (+ ones column for the row-sum trick), built with strided DMAs split across
  both HWDGE queues (explicitly pinned after the xbar chain) + edge memsets.

Per block, queries are tiled in 16 tiles of 128; each tile attends to a
256-key window. Scores are computed transposed (S^T[jj, i]) so probabilities
come out pre-transposed for the PV matmul. Softmax skips the max-subtraction
(scores ~ N(0,1) after the 1/8 scale) and folds the scale into the ScalarE
exp. The band mask is a 0/1 bf16 multiply after exp. Row sums come from the
ones-column: out = (P@[V|1])[:,:64] / (P@[V|1])[:,64].

The ScalarE exp chain is the serial bottleneck; emission is interleaved so
nothing ever stalls it: both heads' QK+exp run back-to-back while V loads,
head 0's PV/normalize work is woven into head 1's QK loop (2-group lag), and
the band masks are split between DVE and the otherwise-idle Pool engine so
DVE can retire normalizes as soon as each PV lands.
"""

import sys

for _p in ("/opt/trn_rl_repo", "/opt/trn_rl_repo/concourse"):
    if _p not in sys.path:
        sys.path.insert(0, _p)

import numpy as np
import ml_dtypes

import concourse.bass as bass
import concourse.mybir as mybir
import concourse.tile as tile
from concourse import bacc
from concourse.bass_utils import run_bass_kernel_spmd
from concourse.tile_rust import add_dep_helper

N_CORES = 8
B, H, S, D = 1, 16, 4096, 64
R = 2                      # dilation rate
NSEQ = S // R              # 2048 per-offset sequence length
HALF = 64                  # window//2
NT = NSEQ // 128           # 16 query tiles per block
HPC = H // N_CORES         # heads per core = 2

F32 = mybir.dt.float32
BF16 = mybir.dt.bfloat16

# xbar piece boundaries per head. QK group g reads kTp columns up to
# 256g+384, i.e. keys up to 256g+320; a piece [0:sp] covers groups with
# 256g+320 <= sp. h0 starts with a small piece so its first groups begin
# as early as possible; h1's pieces land well before its compute anyway.
_QK_PIECES = {0: (512, 1408, NSEQ), 1: (1088, NSEQ)}
# group gated on piece i (piece 0's gate is always at g=0)
_QK_GATES = {0: {1: 1, 2: 5}, 1: {1: 3}}


class _HeadState:
    __slots__ = ("qTs", "kTp", "vsh", "pms", "pos", "out_sb", "dm_gate")

    def __init__(self):
        self.pms = [None] * (NT // 2)
        self.pos = [None] * (NT // 2)
        self.dm_gate = None


def _dummy(nc, dmy, col, a, b, after=None):
    """One PE-proc absorber: a tiny dummy matmul reading cells a/b so the PE
    sequencer observes their DMA-queue semaphores here (<=2 waits). The
    scheduler floats unpinned dummies past the real matmuls (leaving the DMA
    waits guarding nothing), so callers must pin consumers after these via
    add_dep_helper, and `after` keeps the dummy from stalling the PE early."""
    dm = nc.tensor.matmul(dmy[0:1, col : col + 1], lhsT=a, rhs=b,
                          start=True, stop=True)
    if after is not None:
        add_dep_helper(dm.ins, after.ins, reason="absorb order")
    return dm


def _alloc_head(tc, pools):
    nc = tc.nc
    (trans, vpool, *_rest) = pools
    st = _HeadState()
    st.qTs = trans.tile([128, NSEQ], BF16, tag="qTs")
    st.kTp = trans.tile([128, NSEQ + 128], BF16, tag="kTp")
    st.vsh = vpool.tile([128, NT + 1, R, D + 1], BF16, tag="vsh")

    # kTp edge zeros + vsh edges/ones on Pool (keeps both DVE and the DMA
    # paths clear at startup)
    nc.gpsimd.memset(st.kTp[:, 0:HALF], 0.0)
    nc.gpsimd.memset(st.kTp[:, NSEQ + HALF :], 0.0)
    nc.gpsimd.memset(st.vsh[0:64, 0, :, 0:D], 0.0)
    nc.gpsimd.memset(st.vsh[64:128, NT, :, 0:D], 0.0)
    nc.gpsimd.memset(st.vsh[:, :, :, D], 1.0)
    return st


def _load_head_qk(tc, st, qd, kd, h):
    """Xbar transposes, first-needed first. Returns the last transpose."""
    nc = tc.nc
    lo = 0
    last = None
    for sp in _QK_PIECES[h]:
        nc.sync.dma_start_transpose(st.qTs[:, lo:sp], qd[h, lo:sp, :])
        last = nc.sync.dma_start_transpose(
            st.kTp[:, HALF + lo : HALF + sp], kd[h, lo:sp, :]
        )
        lo = sp
    return last


def _load_head_v(tc, st, vd, h, after_xbar):
    """vsh: partitions 64:128 <- tile-aligned rows, 0:64 <- next tile's rows
    (split by dilation offset r to keep each AP 3-dim for the balancer).
    All on the SP queue, FIFO behind the xbar chain — a second HWDGE queue
    stalls its issuing engine on the cross-queue ordering wait, and an
    unpinned queue switch inside the transpose serialization order costs a
    multi-microsecond hop per switch."""
    nc = tc.nc
    for r in range(R):
        nc.sync.dma_start(
            st.vsh[64:128, 0:NT, r, 0:D],
            vd[h, :, 0:64, r].rearrange("t pp d -> pp t d"),
        )
        nc.sync.dma_start(
            st.vsh[0:64, 1 : NT + 1, r, 0:D],
            vd[h, :, 64:128, r].rearrange("t pp d -> pp t d"),
        )


def _emit_qk_group(tc, pools, masks, st, dmy, h, g, ctx):
    """QK scores + exp + band mask for one group of 2 query tiles."""
    nc = tc.nc
    (trans, vpool, ppool, opool, rpool, ps_pool, po_pool, dpool) = pools
    m_first, m_mid, m_last = masks
    qTs, kTp = st.qTs, st.kTp
    pieces = _QK_PIECES[h]
    gates = _QK_GATES[h]
    dc = 8 * (h % 2)

    if g == 0:
        st.dm_gate = _dummy(
            nc, dmy, dc + 0, qTs[0:1, 0:1], qTs[0:1, HALF : HALF + 1],
            after=ctx["last_pe"],
        )
        st.dm_gate = _dummy(
            nc, dmy, dc + 1, kTp[0:1, HALF : HALF + 1],
            kTp[0:1, HALF + 1 : HALF + 2], after=st.dm_gate,
        )
    else:
        for pi, gg in gates.items():
            if g == gg:
                sp = pieces[pi - 1]
                st.dm_gate = _dummy(
                    nc, dmy, dc + 1 + pi, qTs[0:1, sp : sp + 1],
                    kTp[0:1, HALF + sp : HALF + sp + 1], after=ctx["last_pe"],
                )

    ps = ps_pool.tile([128, 2, 4, 128], F32, tag="ps")
    q0 = 256 * g
    for blk, (p0, p1) in enumerate(((0, 64), (64, 128))):
        mm = nc.tensor.matmul(
            ps[:, blk, 0, :],
            lhsT=kTp[p0:p1, q0 : q0 + 128],
            rhs=qTs[p0:p1, q0 : q0 + 128],
            start=True,
            stop=True,
        )
        if st.dm_gate is not None:
            add_dep_helper(mm.ins, st.dm_gate.ins, reason="absorb order")
            st.dm_gate = None
        nc.tensor.matmul(
            ps[:, blk, 1:3, :],
            lhsT=kTp[p0:p1, q0 + 128 : q0 + 256],
            rhs=qTs[p0:p1, q0 : q0 + 256],
            start=True,
            stop=True,
        )
        ctx["last_pe"] = nc.tensor.matmul(
            ps[:, blk, 3, :],
            lhsT=kTp[p0:p1, q0 + 256 : q0 + 384],
            rhs=qTs[p0:p1, q0 + 128 : q0 + 256],
            start=True,
            stop=True,
        )

    # exp((q.k)/8) for both tiles in one ScalarE pass; bf16 out.
    pt = ppool.tile([128, 2, 4, 128], BF16, tag="pt")
    nc.scalar.activation(
        pt[:], ps[:], mybir.ActivationFunctionType.Exp, scale=1.0 / float(D) ** 0.5
    )

    # band mask (0/1 multiply): lo segs keep i<=jj, hi segs keep i>=jj;
    # first/last tiles additionally kill out-of-range keys. Head 1's early
    # groups run on Pool (slack until their PV) so DVE can retire head 0's
    # normalizes during the exp chain instead of after it.
    eng = nc.gpsimd if (h == 1 and g <= 4) else nc.vector
    pm = ppool.tile([128, 2, 4, 128], BF16, tag="pm")
    if 0 < g < NT // 2 - 1:
        eng.tensor_tensor(
            pm[:].rearrange("p b (j c) i -> p (b j) c i", c=2),
            pt[:].rearrange("p b (j c) i -> p (b j) c i", c=2),
            m_mid[:, None, :, :].to_broadcast((128, 4, 2, 128)),
            mybir.AluOpType.mult,
        )
    else:
        for j, t in enumerate((2 * g, 2 * g + 1)):
            m = m_first if t == 0 else (m_last if t == NT - 1 else m_mid)
            eng.tensor_tensor(
                pm[:, :, 2 * j : 2 * j + 2, :],
                pt[:, :, 2 * j : 2 * j + 2, :],
                m[:, None, :, :].to_broadcast((128, 2, 2, 128)),
                mybir.AluOpType.mult,
            )
    st.pms[g] = pm


def _emit_pv_gates(tc, st, dmy, h, ctx):
    nc = tc.nc
    dc = 8 * (h % 2)
    st.dm_gate = _dummy(
        nc, dmy, dc + 5, st.vsh[64:65, 0, 0, 0:1], st.vsh[64:65, 0, 1, 0:1],
        after=ctx["last_pe"],
    )
    st.dm_gate = _dummy(
        nc, dmy, dc + 6, st.vsh[0:1, 1, 0, 0:1], st.vsh[0:1, 1, 1, 0:1],
        after=st.dm_gate,
    )


def _emit_pv_group(tc, pools, st, out, h, g, ctx):
    """PV + row-sum + normalize for one group."""
    nc = tc.nc
    (trans, vpool, ppool, opool, rpool, ps_pool, po_pool, dpool) = pools
    vsh, pm = st.vsh, st.pms[g]

    po = po_pool.tile([128, 2, 2, D + 1], F32, tag="po")
    for j, t in enumerate((2 * g, 2 * g + 1)):
        for blk in range(R):
            mm = nc.tensor.matmul(
                po[:, j, blk, :],
                lhsT=pm[:, blk, 2 * j + 0, :],
                rhs=vsh[:, t, blk, :],
                start=True,
                stop=False,
            )
            if st.dm_gate is not None:
                add_dep_helper(mm.ins, st.dm_gate.ins, reason="absorb order")
                st.dm_gate = None
            ctx["last_pe"] = nc.tensor.matmul(
                po[:, j, blk, :],
                lhsT=pm[:, blk, 2 * j + 1, :],
                rhs=vsh[:, t + 1, blk, :],
                start=False,
                stop=True,
            )
    # normalize both tiles at once: out = po[..., 0:64] / po[..., 64]
    rc = rpool.tile([128, 2, 2], F32, tag="rc")
    nc.vector.reciprocal(rc[:], po[:, :, :, D])
    nc.vector.tensor_tensor(
        st.out_sb[:, 2 * g : 2 * g + 2, :].rearrange("p t (r d) -> p t r d", r=R),
        po[:, :, :, 0:D],
        rc[:, :, :, None].to_broadcast((128, 2, R, D)),
        mybir.AluOpType.mult,
    )

    if g == NT // 2 - 1:
        # single store per head (a store interleaved with the transposes
        # would serialize against them; by now the xbar chain has drained)
        nc.sync.dma_start(
            out[h].rearrange("(t p r) d -> p t (r d)", p=128, r=R),
            st.out_sb[:],
        )


def _build_masks(tc, mpool):
    """Band masks [128, 2(lo|hi), 128].

    Element [jj, c, i]: lo (c=0) keeps i <= jj, hi (c=1) keeps i >= jj.
    t=0 variant also kills jj < 64 in lo (keys < 0); t=NT-1 variant kills
    jj >= 64 in hi (keys >= NSEQ).
    """
    nc = tc.nc
    ge = mybir.AluOpType.is_ge
    tiles = []
    for name in ("m_first", "m_mid", "m_last"):
        m = mpool.tile([128, 2, 128], BF16, tag=name)
        nc.gpsimd.memset(m[:], 1.0)
        # lo: keep jj - i >= 0
        nc.gpsimd.affine_select(
            m[:, 0, :], m[:, 0, :], [[-1, 128]], ge, 0.0,
            base=0, channel_multiplier=1,
        )
        # hi: keep i - jj >= 0
        nc.gpsimd.affine_select(
            m[:, 1, :], m[:, 1, :], [[1, 128]], ge, 0.0,
            base=0, channel_multiplier=-1,
        )
        tiles.append(m)
    m_first, m_mid, m_last = tiles
    # first tile: lo also needs jj >= 64
    nc.gpsimd.affine_select(
        m_first[:, 0, :], m_first[:, 0, :], [[0, 128]], ge, 0.0,
        base=-HALF, channel_multiplier=1,
    )
    # last tile: hi also needs jj <= 63
    nc.gpsimd.affine_select(
        m_last[:, 1, :], m_last[:, 1, :], [[0, 128]], ge, 0.0,
        base=HALF - 1, channel_multiplier=-1,
    )

    # DVE-proc absorber: make the DVE clock observe the final Pool init op
    # here so the first real DVE mask multiply carries only its exp wait
    # (the TensorTensor ISA struct has a single sync-wait slot).
    mdmy = mpool.tile([1, 2], BF16, tag="mdmy")
    nc.vector.tensor_tensor(
        mdmy[0:1, 0:1], m_last[0:1, 0, 0:1], m_last[0:1, 1, 0:1],
        mybir.AluOpType.mult,
    )
    return m_first, m_mid, m_last


def build_bass():
    nc = bacc.Bacc("TRN2", target_bir_lowering=False, debug=False)
    qd = nc.dram_tensor("q", [HPC, NSEQ, R * D], BF16, kind="ExternalInput")
    kd = nc.dram_tensor("k", [HPC, NSEQ, R * D], BF16, kind="ExternalInput")
    vd = nc.dram_tensor("v", [HPC, NT, 128, R, D], BF16, kind="ExternalInput")
    out = nc.dram_tensor("out", [HPC, S, D], F32, kind="ExternalOutput")

    NG = NT // 2
    with tile.TileContext(nc) as tc:
        with (
            tc.tile_pool(name="mpool", bufs=1) as mpool,
            tc.tile_pool(name="trans", bufs=2) as trans,
            tc.tile_pool(name="vpool", bufs=2) as vpool,
            tc.tile_pool(name="ppool", bufs=4 * NG) as ppool,
            tc.tile_pool(name="opool", bufs=2) as opool,
            tc.tile_pool(name="rpool", bufs=8) as rpool,
            tc.tile_pool(name="ps_pool", bufs=2, space="PSUM") as ps_pool,
            tc.tile_pool(name="po_pool", bufs=3, space="PSUM") as po_pool,
            tc.tile_pool(name="dmy_pool", bufs=1, space="PSUM") as dpool,
        ):
            masks = _build_masks(tc, mpool)
            pools = (trans, vpool, ppool, opool, rpool, ps_pool, po_pool, dpool)
            heads = [_alloc_head(tc, pools) for _ in range(HPC)]
            for h in range(HPC):
                out_sb = opool.tile([128, NT, 128], F32, tag="out_sb")
                heads[h].out_sb = out_sb
            # xbar chain first (it serializes against all other DMA), then
            # the V loads ride behind it
            last_xbar = None
            for h in range(HPC):
                last_xbar = _load_head_qk(tc, heads[h], qd, kd, h)
            for h in range(HPC):
                _load_head_v(tc, heads[h], vd, h, last_xbar)

            dmy = dpool.tile([1, 16], F32, tag="dmy")
            ctx = {"last_pe": None}
            # head 0: QK+exp+mask for all groups
            for g in range(NG):
                _emit_qk_group(tc, pools, masks, heads[0], dmy, 0, g, ctx)
            # head 1 QK with head-0 PV woven in at a 2-group lag (v-loads
            # have landed by then; PE never stalls the exp chain)
            for g in range(NG):
                _emit_qk_group(tc, pools, masks, heads[1], dmy, 1, g, ctx)
                if g == 2:
                    _emit_pv_gates(tc, heads[0], dmy, 0, ctx)
                if g >= 2:
                    _emit_pv_group(tc, pools, heads[0], out[:], 0, g - 2, ctx)
            for g in range(NG - 2, NG):
                _emit_pv_group(tc, pools, heads[0], out[:], 0, g, ctx)
            _emit_pv_gates(tc, heads[1], dmy, 1, ctx)
            for g in range(NG):
                _emit_pv_group(tc, pools, heads[1], out[:], 1, g, ctx)
    nc.compile()
    return nc


_NC_CACHE = None


def kernel(q: np.ndarray, k: np.ndarray, v: np.ndarray) -> np.ndarray:
    global _NC_CACHE
    if _NC_CACHE is None:
        _NC_CACHE = build_bass()
    nc = _NC_CACHE

    # host side: one bf16 cast per tensor; all relayout is on-device
    qb = np.asarray(q, dtype=np.float32).reshape(H, NSEQ, R * D).astype(
        ml_dtypes.bfloat16
    )
    kb = np.asarray(k, dtype=np.float32).reshape(H, NSEQ, R * D).astype(
        ml_dtypes.bfloat16
    )
    vb = np.asarray(v, dtype=np.float32).reshape(H, NT, 128, R, D).astype(
        ml_dtypes.bfloat16
    )

    in_maps = []
    for c in range(N_CORES):
        hs = slice(c * HPC, (c + 1) * HPC)
        in_maps.append({"q": qb[hs], "k": kb[hs], "v": vb[hs]})

    res = run_bass_kernel_spmd(nc, in_maps, core_ids=list(range(N_CORES)))
    out = np.empty((B, H, S, D), dtype=np.float32)
    for c in range(N_CORES):
        out[0, c * HPC : (c + 1) * HPC] = res.results[c]["out"]
    return out


# revision 47
# speedup vs baseline: 1.1437x; 1.0645x over previous
"""Dilated attention (banded local-window attention) for Trainium2.

Problem: q,k,v [1, 16, 4096, 64] fp32; dilation r=2, window 128 (band |i-j|<=64
within each of the 2 strided subsequences of length 2048 per head).

Sharding: 16 heads -> 8 cores x 2 heads. Host work is a single fp32->bf16 cast
per tensor (ml_dtypes, ~10ms each); ALL relayout happens on device:

- Q^T/K^T: xbar DMA-transposes turn DRAM-natural [2048, (r d)=128] bf16 into
  SBUF [(r d), 2048] d-major layout; offset r=0 lands on partitions 0:64 and
  r=1 on 64:128 (the dilation de-interleave falls out of the transpose since
  (r d) becomes the partition dim). K^T lands at column offset 64 inside a
  [128, 2176] tile whose edge columns are zeroed. The transposes are SP-issued
  (ACT-issued DmaTranspose corrupts on this runtime) and serialize against
  all other DMA traffic, so they run as one tight chain before the V loads.
- V: the shifted window layout vsh[p, t, r, :64] = v[2*(128t - 64 + p) + r]
  (+ ones column for the row-sum trick), built with strided DMAs split across
  both HWDGE queues (explicitly pinned after the xbar chain) + edge memsets.

Per block, queries are tiled in 16 tiles of 128; each tile attends to a
256-key window. Scores are computed transposed (S^T[jj, i]) so probabilities
come out pre-transposed for the PV matmul. Softmax skips the max-subtraction
(scores ~ N(0,1) after the 1/8 scale) and folds the scale into the ScalarE
exp. The band mask is a 0/1 bf16 multiply after exp. Row sums come from the
ones-column: out = (P@[V|1])[:,:64] / (P@[V|1])[:,64].

The ScalarE exp chain is the serial bottleneck; emission is interleaved so
nothing ever stalls it: both heads' QK+exp run back-to-back while V loads,
head 0's PV/normalize work is woven into head 1's QK loop (2-group lag), and
the band masks are split between DVE and the otherwise-idle Pool engine so
DVE can retire normalizes as soon as each PV lands.
"""

import sys

for _p in ("/opt/trn_rl_repo", "/opt/trn_rl_repo/concourse"):
    if _p not in sys.path:
        sys.path.insert(0, _p)

import numpy as np
import ml_dtypes

import concourse.bass as bass
import concourse.mybir as mybir
import concourse.tile as tile
from concourse import bacc
from concourse.bass_utils import run_bass_kernel_spmd
from concourse.tile_rust import add_dep_helper

N_CORES = 8
B, H, S, D = 1, 16, 4096, 64
R = 2                      # dilation rate
NSEQ = S // R              # 2048 per-offset sequence length
HALF = 64                  # window//2
NT = NSEQ // 128           # 16 query tiles per block
HPC = H // N_CORES         # heads per core = 2

F32 = mybir.dt.float32
BF16 = mybir.dt.bfloat16

# xbar piece boundaries per head. QK group g reads kTp columns up to
# 256g+384, i.e. keys up to 256g+320; a piece [0:sp] covers groups with
# 256g+320 <= sp, so a 1088 split covers g0-g3 and the second piece lands
# (plus its ~3us DMA-sem hop) right as g4 needs it — the exp chain never
# stalls mid-head.
_QK_PIECES = {0: (1088, NSEQ), 1: (1088, NSEQ)}
# group gated on piece i (piece 0's gate is always at g=0)
_QK_GATES = {0: {1: 4}, 1: {1: 4}}


class _HeadState:
    __slots__ = ("qTs", "kTp", "vsh", "pms", "pos", "out_sb", "dm_gate")

    def __init__(self):
        self.pms = [None] * (NT // 2)
        self.pos = [None] * (NT // 2)
        self.dm_gate = None


def _dummy(nc, dmy, col, a, b, after=None):
    """One PE-proc absorber: a tiny dummy matmul reading cells a/b so the PE
    sequencer observes their DMA-queue semaphores here (<=2 waits). The
    scheduler floats unpinned dummies past the real matmuls (leaving the DMA
    waits guarding nothing), so callers must pin consumers after these via
    add_dep_helper, and `after` keeps the dummy from stalling the PE early."""
    dm = nc.tensor.matmul(dmy[0:1, col : col + 1], lhsT=a, rhs=b,
                          start=True, stop=True)
    if after is not None:
        add_dep_helper(dm.ins, after.ins, reason="absorb order")
    return dm


def _alloc_head(tc, pools):
    nc = tc.nc
    (trans, vpool, *_rest) = pools
    st = _HeadState()
    st.qTs = trans.tile([128, NSEQ], BF16, tag="qTs")
    st.kTp = trans.tile([128, NSEQ + 128], BF16, tag="kTp")
    st.vsh = vpool.tile([128, NT + 1, R, D + 1], BF16, tag="vsh")

    # kTp edge zeros + vsh edges/ones on Pool (keeps both DVE and the DMA
    # paths clear at startup)
    nc.gpsimd.memset(st.kTp[:, 0:HALF], 0.0)
    nc.gpsimd.memset(st.kTp[:, NSEQ + HALF :], 0.0)
    nc.gpsimd.memset(st.vsh[0:64, 0, :, 0:D], 0.0)
    nc.gpsimd.memset(st.vsh[64:128, NT, :, 0:D], 0.0)
    nc.gpsimd.memset(st.vsh[:, :, :, D], 1.0)
    return st


def _load_head_qk(tc, st, qd, kd, h):
    """Xbar transposes, first-needed first. Returns the last transpose."""
    nc = tc.nc
    lo = 0
    last = None
    for sp in _QK_PIECES[h]:
        nc.sync.dma_start_transpose(st.qTs[:, lo:sp], qd[h, lo:sp, :])
        last = nc.sync.dma_start_transpose(
            st.kTp[:, HALF + lo : HALF + sp], kd[h, lo:sp, :]
        )
        lo = sp
    return last


def _load_head_v(tc, st, vd, h, after_xbar):
    """vsh: partitions 64:128 <- tile-aligned rows, 0:64 <- next tile's rows
    (split by dilation offset r to keep each AP 3-dim for the balancer).
    All on the SP queue, FIFO behind the xbar chain — a second HWDGE queue
    stalls its issuing engine on the cross-queue ordering wait, and an
    unpinned queue switch inside the transpose serialization order costs a
    multi-microsecond hop per switch."""
    nc = tc.nc
    for r in range(R):
        nc.sync.dma_start(
            st.vsh[64:128, 0:NT, r, 0:D],
            vd[h, :, 0:64, r].rearrange("t pp d -> pp t d"),
        )
        nc.sync.dma_start(
            st.vsh[0:64, 1 : NT + 1, r, 0:D],
            vd[h, :, 64:128, r].rearrange("t pp d -> pp t d"),
        )


def _emit_qk_group(tc, pools, masks, st, dmy, h, g, ctx):
    """QK scores + exp + band mask for one group of 2 query tiles."""
    nc = tc.nc
    (trans, vpool, ppool, opool, rpool, ps_pool, po_pool, dpool) = pools
    m_first, m_mid, m_last = masks
    qTs, kTp = st.qTs, st.kTp
    pieces = _QK_PIECES[h]
    gates = _QK_GATES[h]
    dc = 8 * (h % 2)

    if g == 0:
        st.dm_gate = _dummy(
            nc, dmy, dc + 0, qTs[0:1, 0:1], qTs[0:1, HALF : HALF + 1],
            after=ctx["last_pe"],
        )
        st.dm_gate = _dummy(
            nc, dmy, dc + 1, kTp[0:1, HALF : HALF + 1],
            kTp[0:1, HALF + 1 : HALF + 2], after=st.dm_gate,
        )
    else:
        for pi, gg in gates.items():
            if g == gg:
                sp = pieces[pi - 1]
                st.dm_gate = _dummy(
                    nc, dmy, dc + 1 + pi, qTs[0:1, sp : sp + 1],
                    kTp[0:1, HALF + sp : HALF + sp + 1], after=ctx["last_pe"],
                )

    ps = ps_pool.tile([128, 2, 4, 128], F32, tag="ps")
    q0 = 256 * g
    for blk, (p0, p1) in enumerate(((0, 64), (64, 128))):
        mm = nc.tensor.matmul(
            ps[:, blk, 0, :],
            lhsT=kTp[p0:p1, q0 : q0 + 128],
            rhs=qTs[p0:p1, q0 : q0 + 128],
            start=True,
            stop=True,
        )
        if st.dm_gate is not None:
            add_dep_helper(mm.ins, st.dm_gate.ins, reason="absorb order")
            st.dm_gate = None
        nc.tensor.matmul(
            ps[:, blk, 1:3, :],
            lhsT=kTp[p0:p1, q0 + 128 : q0 + 256],
            rhs=qTs[p0:p1, q0 : q0 + 256],
            start=True,
            stop=True,
        )
        ctx["last_pe"] = nc.tensor.matmul(
            ps[:, blk, 3, :],
            lhsT=kTp[p0:p1, q0 + 256 : q0 + 384],
            rhs=qTs[p0:p1, q0 + 128 : q0 + 256],
            start=True,
            stop=True,
        )

    # exp((q.k)/8) for both tiles in one ScalarE pass; bf16 out.
    pt = ppool.tile([128, 2, 4, 128], BF16, tag="pt")
    nc.scalar.activation(
        pt[:], ps[:], mybir.ActivationFunctionType.Exp, scale=1.0 / float(D) ** 0.5
    )

    # band mask (0/1 multiply): lo segs keep i<=jj, hi segs keep i>=jj;
    # first/last tiles additionally kill out-of-range keys. Head 1's early
    # groups run on Pool (slack until their PV) so DVE can retire head 0's
    # normalizes during the exp chain instead of after it.
    eng = nc.gpsimd if (h == 1 and g <= 4) else nc.vector
    pm = ppool.tile([128, 2, 4, 128], BF16, tag="pm")
    if 0 < g < NT // 2 - 1:
        eng.tensor_tensor(
            pm[:].rearrange("p b (j c) i -> p (b j) c i", c=2),
            pt[:].rearrange("p b (j c) i -> p (b j) c i", c=2),
            m_mid[:, None, :, :].to_broadcast((128, 4, 2, 128)),
            mybir.AluOpType.mult,
        )
    else:
        for j, t in enumerate((2 * g, 2 * g + 1)):
            m = m_first if t == 0 else (m_last if t == NT - 1 else m_mid)
            eng.tensor_tensor(
                pm[:, :, 2 * j : 2 * j + 2, :],
                pt[:, :, 2 * j : 2 * j + 2, :],
                m[:, None, :, :].to_broadcast((128, 2, 2, 128)),
                mybir.AluOpType.mult,
            )
    st.pms[g] = pm


def _emit_pv_gates(tc, st, dmy, h, ctx):
    nc = tc.nc
    dc = 8 * (h % 2)
    st.dm_gate = _dummy(
        nc, dmy, dc + 5, st.vsh[64:65, 0, 0, 0:1], st.vsh[64:65, 0, 1, 0:1],
        after=ctx["last_pe"],
    )
    st.dm_gate = _dummy(
        nc, dmy, dc + 6, st.vsh[0:1, 1, 0, 0:1], st.vsh[0:1, 1, 1, 0:1],
        after=st.dm_gate,
    )


def _emit_pv_group(tc, pools, st, out, h, g, ctx):
    """PV + row-sum + normalize for one group."""
    nc = tc.nc
    (trans, vpool, ppool, opool, rpool, ps_pool, po_pool, dpool) = pools
    vsh, pm = st.vsh, st.pms[g]

    po = po_pool.tile([128, 2, 2, D + 1], F32, tag="po")
    for j, t in enumerate((2 * g, 2 * g + 1)):
        for blk in range(R):
            mm = nc.tensor.matmul(
                po[:, j, blk, :],
                lhsT=pm[:, blk, 2 * j + 0, :],
                rhs=vsh[:, t, blk, :],
                start=True,
                stop=False,
            )
            if st.dm_gate is not None:
                add_dep_helper(mm.ins, st.dm_gate.ins, reason="absorb order")
                st.dm_gate = None
            ctx["last_pe"] = nc.tensor.matmul(
                po[:, j, blk, :],
                lhsT=pm[:, blk, 2 * j + 1, :],
                rhs=vsh[:, t + 1, blk, :],
                start=False,
                stop=True,
            )
    # normalize both tiles at once: out = po[..., 0:64] / po[..., 64]
    rc = rpool.tile([128, 2, 2], F32, tag="rc")
    nc.vector.reciprocal(rc[:], po[:, :, :, D])
    nc.vector.tensor_tensor(
        st.out_sb[:, 2 * g : 2 * g + 2, :].rearrange("p t (r d) -> p t r d", r=R),
        po[:, :, :, 0:D],
        rc[:, :, :, None].to_broadcast((128, 2, R, D)),
        mybir.AluOpType.mult,
    )

    if g == NT // 2 - 1:
        # single store per head (a store interleaved with the transposes
        # would serialize against them; by now the xbar chain has drained)
        nc.sync.dma_start(
            out[h].rearrange("(t p r) d -> p t (r d)", p=128, r=R),
            st.out_sb[:],
        )


def _build_masks(tc, mpool):
    """Band masks [128, 2(lo|hi), 128].

    Element [jj, c, i]: lo (c=0) keeps i <= jj, hi (c=1) keeps i >= jj.
    t=0 variant also kills jj < 64 in lo (keys < 0); t=NT-1 variant kills
    jj >= 64 in hi (keys >= NSEQ).
    """
    nc = tc.nc
    ge = mybir.AluOpType.is_ge
    tiles = []
    for name in ("m_first", "m_mid", "m_last"):
        m = mpool.tile([128, 2, 128], BF16, tag=name)
        nc.gpsimd.memset(m[:], 1.0)
        # lo: keep jj - i >= 0
        nc.gpsimd.affine_select(
            m[:, 0, :], m[:, 0, :], [[-1, 128]], ge, 0.0,
            base=0, channel_multiplier=1,
        )
        # hi: keep i - jj >= 0
        nc.gpsimd.affine_select(
            m[:, 1, :], m[:, 1, :], [[1, 128]], ge, 0.0,
            base=0, channel_multiplier=-1,
        )
        tiles.append(m)
    m_first, m_mid, m_last = tiles
    # first tile: lo also needs jj >= 64
    nc.gpsimd.affine_select(
        m_first[:, 0, :], m_first[:, 0, :], [[0, 128]], ge, 0.0,
        base=-HALF, channel_multiplier=1,
    )
    # last tile: hi also needs jj <= 63
    nc.gpsimd.affine_select(
        m_last[:, 1, :], m_last[:, 1, :], [[0, 128]], ge, 0.0,
        base=HALF - 1, channel_multiplier=-1,
    )

    # DVE-proc absorber: make the DVE clock observe the final Pool init op
    # here so the first real DVE mask multiply carries only its exp wait
    # (the TensorTensor ISA struct has a single sync-wait slot).
    mdmy = mpool.tile([1, 2], BF16, tag="mdmy")
    nc.vector.tensor_tensor(
        mdmy[0:1, 0:1], m_last[0:1, 0, 0:1], m_last[0:1, 1, 0:1],
        mybir.AluOpType.mult,
    )
    return m_first, m_mid, m_last


def build_bass():
    nc = bacc.Bacc("TRN2", target_bir_lowering=False, debug=False)
    qd = nc.dram_tensor("q", [HPC, NSEQ, R * D], BF16, kind="ExternalInput")
    kd = nc.dram_tensor("k", [HPC, NSEQ, R * D], BF16, kind="ExternalInput")
    vd = nc.dram_tensor("v", [HPC, NT, 128, R, D], BF16, kind="ExternalInput")
    out = nc.dram_tensor("out", [HPC, S, D], F32, kind="ExternalOutput")

    NG = NT // 2
    with tile.TileContext(nc) as tc:
        with (
            tc.tile_pool(name="mpool", bufs=1) as mpool,
            tc.tile_pool(name="trans", bufs=2) as trans,
            tc.tile_pool(name="vpool", bufs=2) as vpool,
            tc.tile_pool(name="ppool", bufs=4 * NG) as ppool,
            tc.tile_pool(name="opool", bufs=2) as opool,
            tc.tile_pool(name="rpool", bufs=8) as rpool,
            tc.tile_pool(name="ps_pool", bufs=2, space="PSUM") as ps_pool,
            tc.tile_pool(name="po_pool", bufs=3, space="PSUM") as po_pool,
            tc.tile_pool(name="dmy_pool", bufs=1, space="PSUM") as dpool,
        ):
            masks = _build_masks(tc, mpool)
            pools = (trans, vpool, ppool, opool, rpool, ps_pool, po_pool, dpool)
            heads = [_alloc_head(tc, pools) for _ in range(HPC)]
            for h in range(HPC):
                out_sb = opool.tile([128, NT, 128], F32, tag="out_sb")
                heads[h].out_sb = out_sb
            # xbar chain first (it serializes against all other DMA), then
            # the V loads ride behind it
            last_xbar = None
            for h in range(HPC):
                last_xbar = _load_head_qk(tc, heads[h], qd, kd, h)
            for h in range(HPC):
                _load_head_v(tc, heads[h], vd, h, last_xbar)

            dmy = dpool.tile([1, 16], F32, tag="dmy")
            ctx = {"last_pe": None}
            # head 0: QK+exp+mask for all groups
            for g in range(NG):
                _emit_qk_group(tc, pools, masks, heads[0], dmy, 0, g, ctx)
            # head 1 QK with head-0 PV woven in at a 4-group lag (v-loads
            # plus their DMA-sem hop have landed by then; PE never stalls
            # the exp chain)
            for g in range(NG):
                _emit_qk_group(tc, pools, masks, heads[1], dmy, 1, g, ctx)
                if g == 4:
                    _emit_pv_gates(tc, heads[0], dmy, 0, ctx)
                if g >= 4:
                    _emit_pv_group(tc, pools, heads[0], out[:], 0, g - 4, ctx)
            for g in range(NG - 4, NG):
                _emit_pv_group(tc, pools, heads[0], out[:], 0, g, ctx)
            _emit_pv_gates(tc, heads[1], dmy, 1, ctx)
            for g in range(NG):
                _emit_pv_group(tc, pools, heads[1], out[:], 1, g, ctx)
    nc.compile()
    return nc


_NC_CACHE = None


def kernel(q: np.ndarray, k: np.ndarray, v: np.ndarray) -> np.ndarray:
    global _NC_CACHE
    if _NC_CACHE is None:
        _NC_CACHE = build_bass()
    nc = _NC_CACHE

    # host side: one bf16 cast per tensor; all relayout is on-device
    qb = np.asarray(q, dtype=np.float32).reshape(H, NSEQ, R * D).astype(
        ml_dtypes.bfloat16
    )
    kb = np.asarray(k, dtype=np.float32).reshape(H, NSEQ, R * D).astype(
        ml_dtypes.bfloat16
    )
    vb = np.asarray(v, dtype=np.float32).reshape(H, NT, 128, R, D).astype(
        ml_dtypes.bfloat16
    )

    in_maps = []
    for c in range(N_CORES):
        hs = slice(c * HPC, (c + 1) * HPC)
        in_maps.append({"q": qb[hs], "k": kb[hs], "v": vb[hs]})

    res = run_bass_kernel_spmd(nc, in_maps, core_ids=list(range(N_CORES)))
    out = np.empty((B, H, S, D), dtype=np.float32)
    for c in range(N_CORES):
        out[0, c * HPC : (c + 1) * HPC] = res.results[c]["out"]
    return out


# revision 55
# speedup vs baseline: 1.1953x; 1.0451x over previous
"""Dilated attention (banded local-window attention) for Trainium2.

Problem: q,k,v [1, 16, 4096, 64] fp32; dilation r=2, window 128 (band |i-j|<=64
within each of the 2 strided subsequences of length 2048 per head).

Sharding: 16 heads -> 8 cores x 2 heads. Host work is a single fp32->bf16 cast
per tensor (ml_dtypes, ~10ms each); ALL relayout happens on device:

- Q^T/K^T: xbar DMA-transposes turn DRAM-natural [2048, (r d)=128] bf16 into
  SBUF [(r d), 2048] d-major layout; offset r=0 lands on partitions 0:64 and
  r=1 on 64:128 (the dilation de-interleave falls out of the transpose since
  (r d) becomes the partition dim). K^T lands at column offset 64 inside a
  [128, 2176] tile whose edge columns are zeroed. The transposes are SP-issued
  (ACT-issued DmaTranspose corrupts on this runtime) and serialize against
  all other DMA traffic, so they run as one tight chain before the V loads.
- V: the shifted window layout vsh[p, t, r, :64] = v[2*(128t - 64 + p) + r]
  (+ ones column for the row-sum trick), built with strided DMAs + memsets.

Per block, queries are tiled in 16 tiles of 128; each tile attends to a
256-key window. Scores are computed transposed (S^T[jj, i]) so probabilities
come out pre-transposed for the PV matmul. Softmax skips the max-subtraction
(scores ~ N(0,1) after the 1/8 scale) and folds the scale into the ScalarE
exp. The band mask is a 0/1 bf16 multiply after exp. Row sums come from the
ones-column: out = (P@[V|1])[:,:64] / (P@[V|1])[:,64].

The ScalarE exp chain is the serial bottleneck; emission keeps it fed: both
heads' QK+exp run back-to-back while V loads, head 0's PV work is woven into
head 1's QK loop once the V loads (plus their DMA-sem hop) land, and the
band masks are split between DVE and the otherwise-idle Pool engine. PV
output pairs share one bank-padded PSUM tile so normalize runs at two groups
per reciprocal+multiply, halving the DVE close-out. DMA-queue semaphores are
absorbed by standalone LDWEIGHTS gates (one wait slot each, no PSUM), which
frees the eighth PSUM bank for the paired PV tiles.
"""

import sys

for _p in ("/opt/trn_rl_repo", "/opt/trn_rl_repo/concourse"):
    if _p not in sys.path:
        sys.path.insert(0, _p)

import numpy as np
import ml_dtypes

import concourse.bass as bass
import concourse.mybir as mybir
import concourse.tile as tile
from concourse import bacc
from concourse.bass_utils import run_bass_kernel_spmd
from concourse.tile_rust import add_dep_helper

N_CORES = 8
B, H, S, D = 1, 16, 4096, 64
R = 2                      # dilation rate
NSEQ = S // R              # 2048 per-offset sequence length
HALF = 64                  # window//2
NT = NSEQ // 128           # 16 query tiles per block
HPC = H // N_CORES         # heads per core = 2

F32 = mybir.dt.float32
BF16 = mybir.dt.bfloat16

# xbar piece boundaries per head. QK group g reads kTp columns up to
# 256g+384, i.e. keys up to 256g+320; a piece [0:sp] covers groups with
# 256g+320 <= sp, so a 1088 split covers g0-g3 and the second piece lands
# (plus its ~3us DMA-sem hop) right as g4 needs it.
_QK_PIECES = {0: (1088, NSEQ), 1: (1088, NSEQ)}
_QK_GATES = {0: {1: 4}, 1: {1: 4}}


class _HeadState:
    __slots__ = (
        "qTs", "kTp", "vsh", "pms", "po2", "out_sb", "gate",
        "qk_pieces", "v_dmas",
    )

    def __init__(self):
        self.pms = [None] * (NT // 2)
        self.po2 = None
        self.gate = None


def _gate(nc, dmas, after=None):
    """PE-proc absorber: a wait-table-capable PE nop that observes the given
    DMA instructions' queue semaphores, so the real matmuls never combine
    DMA waits with their steady-state PSUM-WAW waits (MM ISA wait limit is
    2). The scheduler floats unpinned absorbers past the real matmuls, so
    the caller must pin the first consumer after the returned instruction
    via add_dep_helper; `after` keeps the gate from stalling the PE before
    it matters."""
    nop = nc.tensor.nop()
    for dm in dmas:
        add_dep_helper(nop.ins, dm.ins, reason="absorb dma")
    if after is not None:
        add_dep_helper(nop.ins, after.ins, reason="absorb order")
    return nop


def _alloc_head(tc, pools):
    nc = tc.nc
    (trans, vpool, *_rest) = pools
    st = _HeadState()
    st.qTs = trans.tile([128, NSEQ], BF16, tag="qTs")
    st.kTp = trans.tile([128, NSEQ + 128], BF16, tag="kTp")
    st.vsh = vpool.tile([128, NT + 1, R, D + 1], BF16, tag="vsh")

    # kTp edge zeros + vsh edges/ones on Pool (keeps DVE and DMA paths clear)
    nc.gpsimd.memset(st.kTp[:, 0:HALF], 0.0)
    nc.gpsimd.memset(st.kTp[:, NSEQ + HALF :], 0.0)
    nc.gpsimd.memset(st.vsh[0:64, 0, :, 0:D], 0.0)
    nc.gpsimd.memset(st.vsh[64:128, NT, :, 0:D], 0.0)
    nc.gpsimd.memset(st.vsh[:, :, :, D], 1.0)
    return st


def _load_head_qk(tc, st, qd, kd, h):
    """Xbar transposes, first-needed first. Returns per-piece DMA handles."""
    nc = tc.nc
    lo = 0
    pieces = []
    for sp in _QK_PIECES[h]:
        dq = nc.sync.dma_start_transpose(st.qTs[:, lo:sp], qd[h, lo:sp, :])
        dk = nc.sync.dma_start_transpose(
            st.kTp[:, HALF + lo : HALF + sp], kd[h, lo:sp, :]
        )
        pieces.append((dq, dk))
        lo = sp
    return pieces


def _load_head_v(tc, st, vd, h):
    """vsh: partitions 64:128 <- tile-aligned rows, 0:64 <- next tile's rows
    (split by dilation offset r to keep each AP 3-dim for the balancer).
    All on the SP queue, FIFO behind the xbar chain — a second HWDGE queue
    stalls its issuing engine on the cross-queue ordering wait. Returns the
    DMA handles for the PV gate."""
    nc = tc.nc
    dmas = []
    for r in range(R):
        dmas.append(
            nc.sync.dma_start(
                st.vsh[64:128, 0:NT, r, 0:D],
                vd[h, :, 0:64, r].rearrange("t pp d -> pp t d"),
            )
        )
        dmas.append(
            nc.sync.dma_start(
                st.vsh[0:64, 1 : NT + 1, r, 0:D],
                vd[h, :, 64:128, r].rearrange("t pp d -> pp t d"),
            )
        )
    return dmas


def _emit_qk_group(tc, pools, masks, st, h, g, ctx):
    """QK scores + exp + band mask for one group of 2 query tiles."""
    nc = tc.nc
    (trans, vpool, ppool, opool, rpool, ps_pool, po_pool) = pools
    m_first, m_mid, m_last = masks
    qTs, kTp = st.qTs, st.kTp
    pieces = _QK_PIECES[h]
    gates = _QK_GATES[h]

    if g == 0:
        st.gate = _gate(nc, st.qk_pieces[0], after=ctx["last_pe"])
    else:
        for pi, gg in gates.items():
            if g == gg:
                st.gate = _gate(nc, st.qk_pieces[pi], after=ctx["last_pe"])

    ps = ps_pool.tile([128, 2, 4, 128], F32, tag="ps")
    q0 = 256 * g
    for blk, (p0, p1) in enumerate(((0, 64), (64, 128))):
        mm = nc.tensor.matmul(
            ps[:, blk, 0, :],
            lhsT=kTp[p0:p1, q0 : q0 + 128],
            rhs=qTs[p0:p1, q0 : q0 + 128],
            start=True,
            stop=True,
        )
        if st.gate is not None:
            add_dep_helper(mm.ins, st.gate.ins, reason="absorb order")
            st.gate = None
        nc.tensor.matmul(
            ps[:, blk, 1:3, :],
            lhsT=kTp[p0:p1, q0 + 128 : q0 + 256],
            rhs=qTs[p0:p1, q0 : q0 + 256],
            start=True,
            stop=True,
        )
        ctx["last_pe"] = nc.tensor.matmul(
            ps[:, blk, 3, :],
            lhsT=kTp[p0:p1, q0 + 256 : q0 + 384],
            rhs=qTs[p0:p1, q0 + 128 : q0 + 256],
            start=True,
            stop=True,
        )

    # exp((q.k)/8) for both tiles in one ScalarE pass; bf16 out.
    pt = ppool.tile([128, 2, 4, 128], BF16, tag="pt")
    nc.scalar.activation(
        pt[:], ps[:], mybir.ActivationFunctionType.Exp, scale=1.0 / float(D) ** 0.5
    )

    # band mask (0/1 multiply): lo segs keep i<=jj, hi segs keep i>=jj;
    # first/last tiles additionally kill out-of-range keys. Head 1's early
    # groups run on Pool (slack until their PV) so DVE can retire head 0's
    # normalizes during the exp chain instead of after it.
    eng = nc.gpsimd if (h == 1 and g <= 4) else nc.vector
    pm = ppool.tile([128, 2, 4, 128], BF16, tag="pm")
    if 0 < g < NT // 2 - 1:
        eng.tensor_tensor(
            pm[:].rearrange("p b (j c) i -> p (b j) c i", c=2),
            pt[:].rearrange("p b (j c) i -> p (b j) c i", c=2),
            m_mid[:, None, :, :].to_broadcast((128, 4, 2, 128)),
            mybir.AluOpType.mult,
        )
    else:
        for j, t in enumerate((2 * g, 2 * g + 1)):
            m = m_first if t == 0 else (m_last if t == NT - 1 else m_mid)
            eng.tensor_tensor(
                pm[:, :, 2 * j : 2 * j + 2, :],
                pt[:, :, 2 * j : 2 * j + 2, :],
                m[:, None, :, :].to_broadcast((128, 2, 2, 128)),
                mybir.AluOpType.mult,
            )
    st.pms[g] = pm


def _emit_pv_gates(tc, st, ctx):
    nc = tc.nc
    st.gate = _gate(nc, st.v_dmas, after=ctx["last_pe"])


def _emit_pv_group(tc, pools, st, out, h, g, ctx):
    """PV + row-sum for one group; normalize + store fire per PAIR of groups
    (the pair shares one bank-padded PSUM tile, halving DVE close-out ops)."""
    nc = tc.nc
    (trans, vpool, ppool, opool, rpool, ps_pool, po_pool) = pools
    vsh, pm = st.vsh, st.pms[g]
    gg = g % 2

    if gg == 0:
        # values padded to 512B sub-blocks so no PV chunk straddles a bank
        po2 = po_pool.tile(
            [128, 2, 2, 2, D + 1],
            F32,
            tag="po2",
            padded_shape=[128, 2, 2, 2, 128],
        )
        st.po2 = po2
    po2 = st.po2

    for j, t in enumerate((2 * g, 2 * g + 1)):
        for blk in range(R):
            mm = nc.tensor.matmul(
                po2[:, gg, j, blk, :],
                lhsT=pm[:, blk, 2 * j + 0, :],
                rhs=vsh[:, t, blk, :],
                start=True,
                stop=False,
            )
            if st.gate is not None:
                add_dep_helper(mm.ins, st.gate.ins, reason="absorb order")
                st.gate = None
            ctx["last_pe"] = nc.tensor.matmul(
                po2[:, gg, j, blk, :],
                lhsT=pm[:, blk, 2 * j + 1, :],
                rhs=vsh[:, t + 1, blk, :],
                start=False,
                stop=True,
            )

    if gg == 1:
        # normalize 4 tiles at once: out = po[..., 0:64] / po[..., 64]
        p0 = g - 1
        rc = rpool.tile([128, 2, 2, 2], F32, tag="rc")
        nc.vector.reciprocal(rc[:], po2[:, :, :, :, D])
        nc.vector.tensor_tensor(
            st.out_sb[:, 2 * p0 : 2 * p0 + 4, :].rearrange(
                "p (gg j) (r d) -> p gg j r d", gg=2, r=R
            ),
            po2[:, :, :, :, 0:D],
            rc[:, :, :, :, None].to_broadcast((128, 2, 2, R, D)),
            mybir.AluOpType.mult,
        )
        nc.sync.dma_start(
            out[h].rearrange("(t p r) d -> p t (r d)", p=128, r=R)[
                :, 2 * p0 : 2 * p0 + 4, :
            ],
            st.out_sb[:, 2 * p0 : 2 * p0 + 4, :],
        )


def _build_masks(tc, mpool):
    """Band masks [128, 2(lo|hi), 128].

    Element [jj, c, i]: lo (c=0) keeps i <= jj, hi (c=1) keeps i >= jj.
    t=0 variant also kills jj < 64 in lo (keys < 0); t=NT-1 variant kills
    jj >= 64 in hi (keys >= NSEQ).
    """
    nc = tc.nc
    ge = mybir.AluOpType.is_ge
    tiles = []
    for name in ("m_first", "m_mid", "m_last"):
        m = mpool.tile([128, 2, 128], BF16, tag=name)
        nc.gpsimd.memset(m[:], 1.0)
        # lo: keep jj - i >= 0
        nc.gpsimd.affine_select(
            m[:, 0, :], m[:, 0, :], [[-1, 128]], ge, 0.0,
            base=0, channel_multiplier=1,
        )
        # hi: keep i - jj >= 0
        nc.gpsimd.affine_select(
            m[:, 1, :], m[:, 1, :], [[1, 128]], ge, 0.0,
            base=0, channel_multiplier=-1,
        )
        tiles.append(m)
    m_first, m_mid, m_last = tiles
    # first tile: lo also needs jj >= 64
    nc.gpsimd.affine_select(
        m_first[:, 0, :], m_first[:, 0, :], [[0, 128]], ge, 0.0,
        base=-HALF, channel_multiplier=1,
    )
    # last tile: hi also needs jj <= 63
    nc.gpsimd.affine_select(
        m_last[:, 1, :], m_last[:, 1, :], [[0, 128]], ge, 0.0,
        base=HALF - 1, channel_multiplier=-1,
    )

    # DVE-proc absorber: make the DVE clock observe the final Pool init op
    # here so the first real DVE mask multiply carries only its exp wait
    # (the TensorTensor ISA struct has a single sync-wait slot).
    mdmy = mpool.tile([1, 2], BF16, tag="mdmy")
    nc.vector.tensor_tensor(
        mdmy[0:1, 0:1], m_last[0:1, 0, 0:1], m_last[0:1, 1, 0:1],
        mybir.AluOpType.mult,
    )
    return m_first, m_mid, m_last


def build_bass():
    nc = bacc.Bacc("TRN2", target_bir_lowering=False, debug=False)
    qd = nc.dram_tensor("q", [HPC, NSEQ, R * D], BF16, kind="ExternalInput")
    kd = nc.dram_tensor("k", [HPC, NSEQ, R * D], BF16, kind="ExternalInput")
    vd = nc.dram_tensor("v", [HPC, NT, 128, R, D], BF16, kind="ExternalInput")
    out = nc.dram_tensor("out", [HPC, S, D], F32, kind="ExternalOutput")

    NG = NT // 2
    with tile.TileContext(nc) as tc:
        with (
            tc.tile_pool(name="mpool", bufs=1) as mpool,
            tc.tile_pool(name="trans", bufs=2) as trans,
            tc.tile_pool(name="vpool", bufs=2) as vpool,
            tc.tile_pool(name="ppool", bufs=4 * NG) as ppool,
            tc.tile_pool(name="opool", bufs=2) as opool,
            tc.tile_pool(name="rpool", bufs=8) as rpool,
            tc.tile_pool(name="ps_pool", bufs=2, space="PSUM") as ps_pool,
            tc.tile_pool(name="po_pool", bufs=2, space="PSUM") as po_pool,
        ):
            masks = _build_masks(tc, mpool)
            pools = (trans, vpool, ppool, opool, rpool, ps_pool, po_pool)
            heads = [_alloc_head(tc, pools) for _ in range(HPC)]
            for h in range(HPC):
                out_sb = opool.tile([128, NT, 128], F32, tag="out_sb")
                heads[h].out_sb = out_sb
            # xbar chain first (it serializes against all other DMA), then
            # the V loads ride behind it
            for h in range(HPC):
                heads[h].qk_pieces = _load_head_qk(tc, heads[h], qd, kd, h)
            for h in range(HPC):
                heads[h].v_dmas = _load_head_v(tc, heads[h], vd, h)

            ctx = {"last_pe": None}
            for g in range(NG):
                _emit_qk_group(tc, pools, masks, heads[0], 0, g, ctx)
            # head 1 QK with head-0 PV woven in once the v-loads plus their
            # DMA-sem hop have landed
            for g in range(NG):
                _emit_qk_group(tc, pools, masks, heads[1], 1, g, ctx)
                if g == 4:
                    _emit_pv_gates(tc, heads[0], ctx)
                if g >= 4:
                    _emit_pv_group(tc, pools, heads[0], out[:], 0, g - 4, ctx)
            for g in range(NG - 4, NG):
                _emit_pv_group(tc, pools, heads[0], out[:], 0, g, ctx)
            _emit_pv_gates(tc, heads[1], ctx)
            for g in range(NG):
                _emit_pv_group(tc, pools, heads[1], out[:], 1, g, ctx)
    nc.compile()
    return nc


_NC_CACHE = None


def kernel(q: np.ndarray, k: np.ndarray, v: np.ndarray) -> np.ndarray:
    global _NC_CACHE
    if _NC_CACHE is None:
        _NC_CACHE = build_bass()
    nc = _NC_CACHE

    # host side: one bf16 cast per tensor; all relayout is on-device
    qb = np.asarray(q, dtype=np.float32).reshape(H, NSEQ, R * D).astype(
        ml_dtypes.bfloat16
    )
    kb = np.asarray(k, dtype=np.float32).reshape(H, NSEQ, R * D).astype(
        ml_dtypes.bfloat16
    )
    vb = np.asarray(v, dtype=np.float32).reshape(H, NT, 128, R, D).astype(
        ml_dtypes.bfloat16
    )

    in_maps = []
    for c in range(N_CORES):
        hs = slice(c * HPC, (c + 1) * HPC)
        in_maps.append({"q": qb[hs], "k": kb[hs], "v": vb[hs]})

    res = run_bass_kernel_spmd(nc, in_maps, core_ids=list(range(N_CORES)))
    out = np.empty((B, H, S, D), dtype=np.float32)
    for c in range(N_CORES):
        out[0, c * HPC : (c + 1) * HPC] = res.results[c]["out"]
    return out


# revision 56
# speedup vs baseline: 1.2400x; 1.0374x over previous
"""Dilated attention (banded local-window attention) for Trainium2.

Problem: q,k,v [1, 16, 4096, 64] fp32; dilation r=2, window 128 (band |i-j|<=64
within each of the 2 strided subsequences of length 2048 per head).

Sharding: 16 heads -> 8 cores x 2 heads. Host work is a single fp32->bf16 cast
per tensor (ml_dtypes, ~10ms each); ALL relayout happens on device:

- Q^T/K^T: xbar DMA-transposes turn DRAM-natural [2048, (r d)=128] bf16 into
  SBUF [(r d), 2048] d-major layout; offset r=0 lands on partitions 0:64 and
  r=1 on 64:128 (the dilation de-interleave falls out of the transpose since
  (r d) becomes the partition dim). K^T lands at column offset 64 inside a
  [128, 2176] tile whose edge columns are zeroed. The transposes are SP-issued
  (ACT-issued DmaTranspose corrupts on this runtime) and serialize against
  all other DMA traffic, so they run as one tight chain before the V loads.
- V: the shifted window layout vsh[p, t, r, :64] = v[2*(128t - 64 + p) + r]
  (+ ones column for the row-sum trick), built with strided DMAs + memsets.

Per block, queries are tiled in 16 tiles of 128; each tile attends to a
256-key window. Scores are computed transposed (S^T[jj, i]) so probabilities
come out pre-transposed for the PV matmul. Softmax skips the max-subtraction
(scores ~ N(0,1) after the 1/8 scale) and folds the scale into the ScalarE
exp. The band mask is a 0/1 bf16 multiply after exp. Row sums come from the
ones-column: out = (P@[V|1])[:,:64] / (P@[V|1])[:,64].

The ScalarE exp chain is the serial bottleneck; emission keeps it fed: both
heads' QK+exp run back-to-back while V loads, head 0's PV work is woven into
head 1's QK loop once the V loads (plus their DMA-sem hop) land, and the
band masks are split between DVE and the otherwise-idle Pool engine. PV
output pairs share one bank-padded PSUM tile so normalize runs at two groups
per reciprocal+multiply, halving the DVE close-out. DMA-queue semaphores are
absorbed by standalone LDWEIGHTS gates (one wait slot each, no PSUM), which
frees the eighth PSUM bank for the paired PV tiles.
"""

import sys

for _p in ("/opt/trn_rl_repo", "/opt/trn_rl_repo/concourse"):
    if _p not in sys.path:
        sys.path.insert(0, _p)

import numpy as np
import ml_dtypes

import concourse.bass as bass
import concourse.mybir as mybir
import concourse.tile as tile
from concourse import bacc
from concourse.bass_utils import run_bass_kernel_spmd
from concourse.tile_rust import add_dep_helper

N_CORES = 8
B, H, S, D = 1, 16, 4096, 64
R = 2                      # dilation rate
NSEQ = S // R              # 2048 per-offset sequence length
HALF = 64                  # window//2
NT = NSEQ // 128           # 16 query tiles per block
HPC = H // N_CORES         # heads per core = 2

F32 = mybir.dt.float32
BF16 = mybir.dt.bfloat16

# xbar piece boundaries per head. QK group g reads kTp columns up to
# 256g+384, i.e. keys up to 256g+320; a piece [0:sp] covers groups with
# 256g+320 <= sp, so a 1088 split covers g0-g3 and the second piece lands
# (plus its ~3us DMA-sem hop) right as g4 needs it.
_QK_PIECES = {0: (1088, NSEQ), 1: (1088, NSEQ)}
_QK_GATES = {0: {1: 4}, 1: {1: 4}}


class _HeadState:
    __slots__ = (
        "qTs", "kTp", "vsh", "pms", "po2", "out_sb", "gate",
        "qk_pieces", "v_dmas",
    )

    def __init__(self):
        self.pms = [None] * (NT // 2)
        self.po2 = None
        self.gate = None


def _gate(nc, dmas, after=None):
    """PE-proc absorber: a wait-table-capable PE nop that observes the given
    DMA instructions' queue semaphores, so the real matmuls never combine
    DMA waits with their steady-state PSUM-WAW waits (MM ISA wait limit is
    2). The scheduler floats unpinned absorbers past the real matmuls, so
    the caller must pin the first consumer after the returned instruction
    via add_dep_helper; `after` keeps the gate from stalling the PE before
    it matters."""
    nop = nc.tensor.nop()
    for dm in dmas:
        add_dep_helper(nop.ins, dm.ins, reason="absorb dma")
    if after is not None:
        add_dep_helper(nop.ins, after.ins, reason="absorb order")
    return nop


def _alloc_head(tc, pools):
    nc = tc.nc
    (trans, vpool, *_rest) = pools
    st = _HeadState()
    st.qTs = trans.tile([128, NSEQ], BF16, tag="qTs")
    st.kTp = trans.tile([128, NSEQ + 128], BF16, tag="kTp")
    st.vsh = vpool.tile([128, NT + 1, R, D + 1], BF16, tag="vsh")

    # kTp edge zeros + vsh edges/ones on Pool (keeps DVE and DMA paths clear)
    nc.gpsimd.memset(st.kTp[:, 0:HALF], 0.0)
    nc.gpsimd.memset(st.kTp[:, NSEQ + HALF :], 0.0)
    nc.gpsimd.memset(st.vsh[0:64, 0, :, 0:D], 0.0)
    nc.gpsimd.memset(st.vsh[64:128, NT, :, 0:D], 0.0)
    nc.gpsimd.memset(st.vsh[:, :, :, D], 1.0)
    return st


def _load_head_qk(tc, st, qd, kd, h):
    """Xbar transposes, first-needed first. Returns per-piece DMA handles."""
    nc = tc.nc
    lo = 0
    pieces = []
    for sp in _QK_PIECES[h]:
        dq = nc.sync.dma_start_transpose(st.qTs[:, lo:sp], qd[h, lo:sp, :])
        dk = nc.sync.dma_start_transpose(
            st.kTp[:, HALF + lo : HALF + sp], kd[h, lo:sp, :]
        )
        pieces.append((dq, dk))
        lo = sp
    return pieces


def _load_head_v(tc, st, vd, h):
    """vsh: partitions 64:128 <- tile-aligned rows, 0:64 <- next tile's rows
    (split by dilation offset r to keep each AP 3-dim for the balancer).
    All on the SP queue, FIFO behind the xbar chain — a second HWDGE queue
    stalls its issuing engine on the cross-queue ordering wait. Returns the
    DMA handles for the PV gate."""
    nc = tc.nc
    dmas = []
    for r in range(R):
        dmas.append(
            nc.sync.dma_start(
                st.vsh[64:128, 0:NT, r, 0:D],
                vd[h, :, 0:64, r].rearrange("t pp d -> pp t d"),
            )
        )
        dmas.append(
            nc.sync.dma_start(
                st.vsh[0:64, 1 : NT + 1, r, 0:D],
                vd[h, :, 64:128, r].rearrange("t pp d -> pp t d"),
            )
        )
    return dmas


def _emit_qk_group(tc, pools, masks, st, h, g, ctx):
    """QK scores + exp + band mask for one group of 2 query tiles."""
    nc = tc.nc
    (trans, vpool, ppool, opool, rpool, ps_pool, po_pool) = pools
    m_first, m_mid, m_last = masks
    qTs, kTp = st.qTs, st.kTp
    pieces = _QK_PIECES[h]
    gates = _QK_GATES[h]

    if g == 0:
        st.gate = _gate(nc, st.qk_pieces[0], after=ctx["last_pe"])
    else:
        for pi, gg in gates.items():
            if g == gg:
                st.gate = _gate(nc, st.qk_pieces[pi], after=ctx["last_pe"])

    ps = ps_pool.tile([128, 2, 4, 128], F32, tag="ps")
    q0 = 256 * g
    for blk, (p0, p1) in enumerate(((0, 64), (64, 128))):
        mm = nc.tensor.matmul(
            ps[:, blk, 0, :],
            lhsT=kTp[p0:p1, q0 : q0 + 128],
            rhs=qTs[p0:p1, q0 : q0 + 128],
            start=True,
            stop=True,
        )
        if st.gate is not None:
            add_dep_helper(mm.ins, st.gate.ins, reason="absorb order")
            st.gate = None
        nc.tensor.matmul(
            ps[:, blk, 1:3, :],
            lhsT=kTp[p0:p1, q0 + 128 : q0 + 256],
            rhs=qTs[p0:p1, q0 : q0 + 256],
            start=True,
            stop=True,
        )
        ctx["last_pe"] = nc.tensor.matmul(
            ps[:, blk, 3, :],
            lhsT=kTp[p0:p1, q0 + 256 : q0 + 384],
            rhs=qTs[p0:p1, q0 + 128 : q0 + 256],
            start=True,
            stop=True,
        )

    # exp((q.k)/8) for both tiles in one ScalarE pass; bf16 out.
    pt = ppool.tile([128, 2, 4, 128], BF16, tag="pt")
    nc.scalar.activation(
        pt[:], ps[:], mybir.ActivationFunctionType.Exp, scale=1.0 / float(D) ** 0.5
    )

    # band mask (0/1 multiply): lo segs keep i<=jj, hi segs keep i>=jj;
    # first/last tiles additionally kill out-of-range keys. Head 1's early
    # groups run on Pool (slack until their PV) so DVE can retire head 0's
    # normalizes during the exp chain instead of after it.
    eng = nc.gpsimd if (h == 1 and g <= 4) else nc.vector
    pm = ppool.tile([128, 2, 4, 128], BF16, tag="pm")
    if 0 < g < NT // 2 - 1:
        eng.tensor_tensor(
            pm[:].rearrange("p b (j c) i -> p (b j) c i", c=2),
            pt[:].rearrange("p b (j c) i -> p (b j) c i", c=2),
            m_mid[:, None, :, :].to_broadcast((128, 4, 2, 128)),
            mybir.AluOpType.mult,
        )
    else:
        for j, t in enumerate((2 * g, 2 * g + 1)):
            m = m_first if t == 0 else (m_last if t == NT - 1 else m_mid)
            eng.tensor_tensor(
                pm[:, :, 2 * j : 2 * j + 2, :],
                pt[:, :, 2 * j : 2 * j + 2, :],
                m[:, None, :, :].to_broadcast((128, 2, 2, 128)),
                mybir.AluOpType.mult,
            )
    st.pms[g] = pm


def _emit_pv_gates(tc, st, ctx):
    nc = tc.nc
    st.gate = _gate(nc, st.v_dmas, after=ctx["last_pe"])


def _emit_pv_group(tc, pools, st, out, h, g, ctx):
    """PV + row-sum for one group; normalize + store fire per PAIR of groups
    (the pair shares one bank-padded PSUM tile, halving DVE close-out ops)."""
    nc = tc.nc
    (trans, vpool, ppool, opool, rpool, ps_pool, po_pool) = pools
    vsh, pm = st.vsh, st.pms[g]
    gg = g % 2

    if gg == 0:
        # values padded to 512B sub-blocks so no PV chunk straddles a bank
        po2 = po_pool.tile(
            [128, 2, 2, 2, D + 1],
            F32,
            tag="po2",
            padded_shape=[128, 2, 2, 2, 128],
        )
        st.po2 = po2
    po2 = st.po2

    for j, t in enumerate((2 * g, 2 * g + 1)):
        for blk in range(R):
            mm = nc.tensor.matmul(
                po2[:, gg, j, blk, :],
                lhsT=pm[:, blk, 2 * j + 0, :],
                rhs=vsh[:, t, blk, :],
                start=True,
                stop=False,
            )
            if st.gate is not None:
                add_dep_helper(mm.ins, st.gate.ins, reason="absorb order")
                st.gate = None
            ctx["last_pe"] = nc.tensor.matmul(
                po2[:, gg, j, blk, :],
                lhsT=pm[:, blk, 2 * j + 1, :],
                rhs=vsh[:, t + 1, blk, :],
                start=False,
                stop=True,
            )

    if gg == 1:
        # normalize 4 tiles at once: out = po[..., 0:64] / po[..., 64]
        p0 = g - 1
        rc = rpool.tile([128, 2, 2, 2], F32, tag="rc")
        nc.vector.reciprocal(rc[:], po2[:, :, :, :, D])
        nc.vector.tensor_tensor(
            st.out_sb[:, 2 * p0 : 2 * p0 + 4, :].rearrange(
                "p (gg j) (r d) -> p gg j r d", gg=2, r=R
            ),
            po2[:, :, :, :, 0:D],
            rc[:, :, :, :, None].to_broadcast((128, 2, 2, R, D)),
            mybir.AluOpType.mult,
        )
        nc.sync.dma_start(
            out[h].rearrange("(t p r) d -> p t (r d)", p=128, r=R)[
                :, 2 * p0 : 2 * p0 + 4, :
            ],
            st.out_sb[:, 2 * p0 : 2 * p0 + 4, :],
        )


def _build_masks(tc, mpool):
    """Band masks [128, 2(lo|hi), 128].

    Element [jj, c, i]: lo (c=0) keeps i <= jj, hi (c=1) keeps i >= jj.
    t=0 variant also kills jj < 64 in lo (keys < 0); t=NT-1 variant kills
    jj >= 64 in hi (keys >= NSEQ).
    """
    nc = tc.nc
    ge = mybir.AluOpType.is_ge
    tiles = []
    for name in ("m_first", "m_mid", "m_last"):
        m = mpool.tile([128, 2, 128], BF16, tag=name)
        nc.gpsimd.memset(m[:], 1.0)
        # lo: keep jj - i >= 0
        nc.gpsimd.affine_select(
            m[:, 0, :], m[:, 0, :], [[-1, 128]], ge, 0.0,
            base=0, channel_multiplier=1,
        )
        # hi: keep i - jj >= 0
        nc.gpsimd.affine_select(
            m[:, 1, :], m[:, 1, :], [[1, 128]], ge, 0.0,
            base=0, channel_multiplier=-1,
        )
        tiles.append(m)
    m_first, m_mid, m_last = tiles
    # first tile: lo also needs jj >= 64
    nc.gpsimd.affine_select(
        m_first[:, 0, :], m_first[:, 0, :], [[0, 128]], ge, 0.0,
        base=-HALF, channel_multiplier=1,
    )
    # last tile: hi also needs jj <= 63
    nc.gpsimd.affine_select(
        m_last[:, 1, :], m_last[:, 1, :], [[0, 128]], ge, 0.0,
        base=HALF - 1, channel_multiplier=-1,
    )

    # DVE-proc absorber: make the DVE clock observe the final Pool init op
    # here so the first real DVE mask multiply carries only its exp wait
    # (the TensorTensor ISA struct has a single sync-wait slot).
    mdmy = mpool.tile([1, 2], BF16, tag="mdmy")
    nc.vector.tensor_tensor(
        mdmy[0:1, 0:1], m_last[0:1, 0, 0:1], m_last[0:1, 1, 0:1],
        mybir.AluOpType.mult,
    )
    return m_first, m_mid, m_last


def build_bass():
    nc = bacc.Bacc("TRN2", target_bir_lowering=False, debug=False)
    qd = nc.dram_tensor("q", [HPC, NSEQ, R * D], BF16, kind="ExternalInput")
    kd = nc.dram_tensor("k", [HPC, NSEQ, R * D], BF16, kind="ExternalInput")
    vd = nc.dram_tensor("v", [HPC, NT, 128, R, D], BF16, kind="ExternalInput")
    out = nc.dram_tensor("out", [HPC, S, D], F32, kind="ExternalOutput")

    NG = NT // 2
    with tile.TileContext(nc) as tc:
        with (
            tc.tile_pool(name="mpool", bufs=1) as mpool,
            tc.tile_pool(name="trans", bufs=2) as trans,
            tc.tile_pool(name="vpool", bufs=2) as vpool,
            tc.tile_pool(name="ppool", bufs=4 * NG) as ppool,
            tc.tile_pool(name="opool", bufs=2) as opool,
            tc.tile_pool(name="rpool", bufs=8) as rpool,
            tc.tile_pool(name="ps_pool", bufs=2, space="PSUM") as ps_pool,
            tc.tile_pool(name="po_pool", bufs=2, space="PSUM") as po_pool,
        ):
            masks = _build_masks(tc, mpool)
            pools = (trans, vpool, ppool, opool, rpool, ps_pool, po_pool)
            heads = [_alloc_head(tc, pools) for _ in range(HPC)]
            for h in range(HPC):
                out_sb = opool.tile([128, NT, 128], F32, tag="out_sb")
                heads[h].out_sb = out_sb
            # xbar chain first (it serializes against all other DMA), then
            # the V loads ride behind it
            for h in range(HPC):
                heads[h].qk_pieces = _load_head_qk(tc, heads[h], qd, kd, h)
            for h in range(HPC):
                heads[h].v_dmas = _load_head_v(tc, heads[h], vd, h)

            ctx = {"last_pe": None}
            # all QK+exp+mask first: the exp chain is the serial bottleneck
            # and the PV gate (which stalls PE until the V loads' DMA-sem
            # hop lands, ~mid-chain) must sit after every QK so it cannot
            # delay an exp; the PE is fast enough to rip through all PVs
            # afterwards while DVE retires paired normalizes in its wake
            for h in range(HPC):
                for g in range(NG):
                    _emit_qk_group(tc, pools, masks, heads[h], h, g, ctx)
            for h in range(HPC):
                _emit_pv_gates(tc, heads[h], ctx)
                for g in range(NG):
                    _emit_pv_group(tc, pools, heads[h], out[:], h, g, ctx)
    nc.compile()
    return nc


_NC_CACHE = None


def kernel(q: np.ndarray, k: np.ndarray, v: np.ndarray) -> np.ndarray:
    global _NC_CACHE
    if _NC_CACHE is None:
        _NC_CACHE = build_bass()
    nc = _NC_CACHE

    # host side: one bf16 cast per tensor; all relayout is on-device
    qb = np.asarray(q, dtype=np.float32).reshape(H, NSEQ, R * D).astype(
        ml_dtypes.bfloat16
    )
    kb = np.asarray(k, dtype=np.float32).reshape(H, NSEQ, R * D).astype(
        ml_dtypes.bfloat16
    )
    vb = np.asarray(v, dtype=np.float32).reshape(H, NT, 128, R, D).astype(
        ml_dtypes.bfloat16
    )

    in_maps = []
    for c in range(N_CORES):
        hs = slice(c * HPC, (c + 1) * HPC)
        in_maps.append({"q": qb[hs], "k": kb[hs], "v": vb[hs]})

    res = run_bass_kernel_spmd(nc, in_maps, core_ids=list(range(N_CORES)))
    out = np.empty((B, H, S, D), dtype=np.float32)
    for c in range(N_CORES):
        out[0, c * HPC : (c + 1) * HPC] = res.results[c]["out"]
    return out
